# revision 4
# baseline (speedup 1.0000x reference)
"""Causal self-attention (B=2, S=4096, D=512, H=8) on 8 Trainium2 NeuronCores.

Sharding: tensor-parallel over heads. Core h computes head h for both batch
elements: QKV projections for its head, causal flash attention, and its
partial (unnormalized) o_proj contribution y_h = U_h @ Wo[h*64:(h+1)*64, :]
plus the per-query softmax denominators L_h. The host computes
sum_h(y_h / L_h) + bo.

v2 layout (hd = 64, S = 4096, 32 k-tiles of 128 per batch):
  - All 16 xt tiles [128, 4, 512] (bf16, host-pretransposed x) are resident
    in SBUF and prefetched on the Sync queue at t0.
  - Projections per 512-token block: Wq -> q_ps[0:64], [Wk|Wv] -> kv_ps
    (K.T rows 0:64, V.T rows 64:128). Bias-adds write bf16 Q.T/K.T straight
    to partition-base-0 tiles (no duplication / zero-padding DMAs); V.T is
    PE-transposed (identity at base 64) into V-natural blocks vp=[V|ones].
  - Scores: K=64 matmuls lhsT=K.T[64,128], rhs=Q.T[64,512] -> st psum
    [128, 2, 512]; one ACT exp (scale) -> P.T bf16; diagonal chunks get a
    0/1 causal mask multiply on DVE.
  - AV (bf16): U'[65, 512] += V'_kt.T @ P.T_kt; row 64 accumulates L.
  - o_proj: K=65 matmuls lhsT=U'[65,128] (fp32r), rhs=Wo_h' [65,512]
    (row 64 zeroed) -> y [128, 512] fp32, cast bf16, DMA'd out with L.
  - Engine split: PE matmuls; ACT exp + U' psum->sbuf copies; DVE bias-adds
    + causal masks; Pool (gpsimd) vp/y copies + all output DMAs (software
    DGE); Sync all input DMAs. Emission is software-pipelined: the next
    block's projection matmuls are issued inside the previous block's last
    exp window to keep the PE p-state hot.
"""

import sys

for _p in ("/opt/trn_rl_repo", "/root/.axon_site/_ro/trn_rl_repo"):
    if _p not in sys.path:
        sys.path.insert(0, _p)

import numpy as np

import concourse.bass as bass
import concourse.mybir as mybir
import concourse.tile as tile
from concourse import bacc
from concourse.bass_utils import run_bass_kernel_spmd

B = 2
S = 4096
D = 512
H = 8
HD = 64
TOK = B * S          # 8192
NKT = S // 128       # 32 k-tiles per batch
NBLK = 16            # 512-token blocks over both batches
SCALE = HD ** -0.5

F32 = mybir.dt.float32
F32R = mybir.dt.float32r
BF16 = mybir.dt.bfloat16

_CACHE = {}


def _build():
    nc = bacc.Bacc("TRN2", target_bir_lowering=False, debug=False, num_devices=8)

    xt_d = nc.dram_tensor("xt", [D, TOK], BF16, kind="ExternalInput")
    wq_d = nc.dram_tensor("wq", [D, HD], BF16, kind="ExternalInput")
    wkv_d = nc.dram_tensor("wkv", [D, 128], BF16, kind="ExternalInput")
    wo_d = nc.dram_tensor("wo", [65, D], F32R, kind="ExternalInput")
    bq_d = nc.dram_tensor("bq", [HD, 1], F32, kind="ExternalInput")
    bkv_d = nc.dram_tensor("bkv", [128, 1], F32, kind="ExternalInput")
    mask_d = nc.dram_tensor("mask", [128, 4, 512], BF16, kind="ExternalInput")
    identb_d = nc.dram_tensor("identb", [128, 64], BF16, kind="ExternalInput")
    onesb_d = nc.dram_tensor("onesb", [128, NKT], BF16, kind="ExternalInput")
    y_d = nc.dram_tensor("y", [TOK, D], BF16, kind="ExternalOutput")
    l_d = nc.dram_tensor("l", [TOK], F32R, kind="ExternalOutput")

    xt_r = xt_d.ap().rearrange("(c p) t -> p c t", p=128)      # [128, 4, 8192]
    wq_r = wq_d.ap().rearrange("(c p) m -> p c m", p=128)      # [128, 4, 64]
    wkv_r = wkv_d.ap().rearrange("(c p) m -> p c m", p=128)    # [128, 4, 128]

    blocks = [(b, tb) for b in range(B) for tb in range(8)]

    with tile.TileContext(nc) as tc:
        import contextlib

        with contextlib.ExitStack() as ctx:
            singles = ctx.enter_context(tc.tile_pool(name="singles", bufs=1))
            ptpool = ctx.enter_context(tc.tile_pool(name="pt", bufs=4))
            upool = ctx.enter_context(tc.tile_pool(name="usb", bufs=2))
            ypool = ctx.enter_context(tc.tile_pool(name="ysb", bufs=4))
            vtpool = ctx.enter_context(tc.tile_pool(name="vt", bufs=2))

            ps_st = ctx.enter_context(
                tc.tile_pool(name="ps_st", bufs=2, space="PSUM")
            )
            ps_u = ctx.enter_context(tc.tile_pool(name="ps_u", bufs=1, space="PSUM"))
            ps_misc = ctx.enter_context(
                tc.tile_pool(name="ps_misc", bufs=3, space="PSUM")
            )

            # --- constants / weights (issue order = load order on Sync) ---
            wq_sb = singles.tile([128, 4, HD], BF16)
            wkv_sb = singles.tile([128, 4, 128], BF16)
            bq_sb = singles.tile([HD, 1], F32)
            bkv_sb = singles.tile([128, 1], F32)
            nc.sync.dma_start(out=wq_sb, in_=wq_r)
            nc.sync.dma_start(out=wkv_sb, in_=wkv_r)
            nc.sync.dma_start(out=bq_sb, in_=bq_d.ap())
            nc.sync.dma_start(out=bkv_sb, in_=bkv_d.ap())

            xts = [
                singles.tile([128, 4, 512], BF16, tag=f"xt{i}", name=f"xt{i}")
                for i in range(NBLK)
            ]
            nc.sync.dma_start(out=xts[0], in_=xt_r[:, :, 0:512])

            mask_sb = singles.tile([128, 4, 512], BF16)
            wo_sb = singles.tile([65, D], F32R)
            identb = singles.tile([128, HD], BF16)
            nc.sync.dma_start(out=mask_sb, in_=mask_d.ap())
            nc.sync.dma_start(out=wo_sb, in_=wo_d.ap())
            nc.sync.dma_start(out=identb, in_=identb_d.ap())

            # --- persistent per-batch activation buffers ---------------
            qt = [
                singles.tile([HD, S], BF16, tag=f"qt_{b}", name=f"qt_{b}")
                for b in range(B)
            ]
            kt = [
                singles.tile([HD, S], BF16, tag=f"kt_{b}", name=f"kt_{b}")
                for b in range(B)
            ]
            vp = [
                singles.tile([128, NKT * 65], BF16, tag=f"vp_{b}", name=f"vp_{b}")
                for b in range(B)
            ]
            for b in range(B):
                nc.sync.dma_start(
                    out=vp[b].rearrange("p (t c) -> p t c", c=65)[:, :, 64:65],
                    in_=onesb_d.ap().rearrange("p (t c) -> p t c", c=1),
                )
            for i in range(1, NBLK):
                b, tb = blocks[i]
                t0 = b * S + tb * 512
                nc.sync.dma_start(out=xts[i], in_=xt_r[:, :, t0 : t0 + 512])

            def proj_mm(i):
                """Projection matmuls (PE only) for block i."""
                b, tb = blocks[i]
                xt_sb = xts[i]
                q_ps = ps_misc.tile([128, 512], F32, tag="m")
                for c in range(4):
                    nc.tensor.matmul(
                        q_ps[0:HD, :],
                        wq_sb[:, c, :],
                        xt_sb[:, c, :],
                        start=(c == 0),
                        stop=(c == 3),
                    )
                kv_ps = ps_misc.tile([128, 512], F32, tag="m")
                for c in range(4):
                    nc.tensor.matmul(
                        kv_ps,
                        wkv_sb[:, c, :],
                        xt_sb[:, c, :],
                        start=(c == 0),
                        stop=(c == 3),
                    )
                return q_ps, kv_ps

            def proj_adds(i, q_ps, kv_ps):
                """Bias-adds (DVE); returns vt_sb for the transpose step."""
                b, tb = blocks[i]
                cols = slice(tb * 512, (tb + 1) * 512)
                # V first: its transposes unblock the PE soonest.
                vt_sb = vtpool.tile([128, 512], BF16, tag="vt")
                nc.vector.tensor_scalar_add(
                    vt_sb[64:128, :], kv_ps[64:128, :], bkv_sb[64:128, 0:1]
                )
                nc.vector.tensor_scalar_add(
                    qt[b][:, cols], q_ps[0:HD, :], bq_sb[:, 0:1]
                )
                nc.vector.tensor_scalar_add(
                    kt[b][:, cols], kv_ps[0:HD, :], bkv_sb[0:HD, 0:1]
                )
                return vt_sb

            def proj_vp(i, vt_sb):
                """V transposes (PE) + vp copies (DVE)."""
                b, tb = blocks[i]
                for j in range(4):
                    ktile = tb * 4 + j
                    vtr_ps = ps_misc.tile([128, HD], BF16, tag="m")
                    nc.tensor.transpose(
                        vtr_ps,
                        vt_sb[64:128, j * 128 : (j + 1) * 128],
                        identb[64:128, :],
                    )
                    nc.vector.tensor_copy(
                        vp[b][:, ktile * 65 : ktile * 65 + 64], vtr_ps
                    )

            def make_emit_av(b, u_ps, n_chunks):
                def emit_av(pt, j):
                    for j2 in range(2):
                        ktile = 2 * j + j2
                        nc.tensor.matmul(
                            u_ps,
                            vp[b][:, ktile * 65 : ktile * 65 + 65],
                            pt[:, j2, :],
                            start=(ktile == 0),
                            stop=(ktile == 2 * n_chunks - 1),
                            skip_group_check=True,
                        )

                return emit_av

            def attn_chunks(i):
                """ST/exp/mask chunks + all AVs except the last pair."""
                b, qb = blocks[i]
                q0 = qb * 512
                u_ps = ps_u.tile([65, 512], F32, tag="u")
                n_chunks = 2 * (qb + 1)  # chunks of 2 k-tiles
                emit_av = make_emit_av(b, u_ps, n_chunks)

                prev_pt = None
                for j in range(n_chunks):
                    st = ps_st.tile([128, 2, 512], F32, tag="st")
                    for j2 in range(2):
                        ktile = 2 * j + j2
                        nc.tensor.matmul(
                            st[:, j2, :],
                            kt[b][:, ktile * 128 : (ktile + 1) * 128],
                            qt[b][:, q0 : q0 + 512],
                            start=True,
                            stop=True,
                        )
                    pt = ptpool.tile([128, 2, 512], BF16, tag="pt")
                    nc.scalar.activation(
                        pt, st, mybir.ActivationFunctionType.Exp, scale=SCALE
                    )
                    if j >= n_chunks - 2:  # diagonal chunks: causal mask
                        d0 = (j % 2) * 2
                        nc.vector.tensor_mul(pt, pt, mask_sb[:, d0 : d0 + 2, :])
                    if prev_pt is not None:
                        emit_av(prev_pt, j - 1)
                    prev_pt = pt
                return u_ps, prev_pt, emit_av, n_chunks

            def attn_tail_mm(i, u_ps, last_pt, emit_av, n_chunks):
                """Last AV pair, U' drain (ACT), o_proj matmuls, L out."""
                b, qb = blocks[i]
                emit_av(last_pt, n_chunks - 1)

                u_sb = upool.tile([65, 512], F32R, tag="u")
                nc.scalar.copy(u_sb, u_ps)

                row0 = b * S + qb * 512
                nc.gpsimd.dma_start(
                    out=l_d.ap()[row0 : row0 + 512].rearrange("(p c) -> p c", p=1),
                    in_=u_sb[64:65, :],
                )

                # y = U'.T @ Wo_h' (unnormalized); K=65, wo row 64 = 0
                y_pss = []
                for j2 in range(4):
                    y_ps = ps_misc.tile([128, 512], F32, tag="m")
                    nc.tensor.matmul(
                        y_ps,
                        u_sb[:, j2 * 128 : (j2 + 1) * 128],
                        wo_sb,
                        start=True,
                        stop=True,
                    )
                    y_pss.append(y_ps)
                return row0, y_pss

            def attn_tail_out(row0, y_pss):
                """y psum->sbuf casts (DVE) + output DMAs (Sync)."""
                for j2, y_ps in enumerate(y_pss):
                    y_sb = ypool.tile([128, 512], BF16, tag="y")
                    nc.vector.tensor_copy(y_sb, y_ps)
                    r0 = row0 + j2 * 128
                    nc.sync.dma_start(out=y_d.ap()[r0 : r0 + 128, :], in_=y_sb)

            # Software pipeline: proj MMs of block i+1 land inside block i's
            # last exp window; block i's tail (o_proj etc.) follows them.
            q_ps, kv_ps = proj_mm(0)
            proj_vp(0, proj_adds(0, q_ps, kv_ps))
            for i in range(NBLK):
                chunk_state = attn_chunks(i)
                if i + 1 < NBLK:
                    q_ps, kv_ps = proj_mm(i + 1)
                row0, y_pss = attn_tail_mm(i, *chunk_state)
                if i + 1 < NBLK:
                    vt_sb = proj_adds(i + 1, q_ps, kv_ps)
                    if blocks[i + 1][1] == 0:
                        # batch start: attn(i+1) needs its own vp tiles at
                        # chunk 0 — emit them ahead of the y drains.
                        proj_vp(i + 1, vt_sb)
                        attn_tail_out(row0, y_pss)
                    else:
                        attn_tail_out(row0, y_pss)
                        proj_vp(i + 1, vt_sb)
                else:
                    attn_tail_out(row0, y_pss)

    nc.compile()
    return nc


def _prep_inputs(x, Wq, bq, Wk, bk, Wv, bv, Wo, bo):
    import ml_dtypes

    bf16 = ml_dtypes.bfloat16
    xt = np.ascontiguousarray(x.reshape(TOK, D).T).astype(bf16)
    mask = np.zeros((128, 4, 512), dtype=np.float32)
    p = np.arange(128)[:, None]
    c = np.arange(512)[None, :]
    for d in range(4):
        mask[:, d, :] = (p + 128 * d <= c).astype(np.float32)
    mask = mask.astype(bf16)
    identb = np.zeros((128, 64), dtype=np.float32)
    identb[64:128, :] = np.eye(64, dtype=np.float32)
    identb = identb.astype(bf16)
    onesb = np.ones((128, NKT), dtype=np.float32).astype(bf16)

    in_maps = []
    for h in range(H):
        hs = slice(h * HD, (h + 1) * HD)
        wo_h = np.concatenate(
            [Wo[hs, :], np.zeros((1, D), dtype=np.float32)], axis=0
        ).astype(np.float32)
        in_maps.append(
            {
                "xt": xt,
                "wq": np.ascontiguousarray(Wq[:, hs]).astype(bf16),
                "wkv": np.ascontiguousarray(
                    np.concatenate([Wk[:, hs], Wv[:, hs]], axis=1)
                ).astype(bf16),
                "wo": wo_h,
                "bq": bq[hs].reshape(HD, 1).astype(np.float32),
                "bkv": np.concatenate([bk[hs], bv[hs]]).reshape(128, 1).astype(
                    np.float32
                ),
                "mask": mask,
                "identb": identb,
                "onesb": onesb,
            }
        )
    return in_maps


def _install_ntff_hook():
    """Register the axon NTFF profiling hook (test-only plumbing)."""
    import types

    try:
        from antenv.axon_hooks import set_axon_ntff_profile_hook  # noqa: F401
    except ImportError:
        m = types.ModuleType("antenv.axon_hooks")
        m._HOOK = None
        m.set_axon_ntff_profile_hook = lambda h: setattr(m, "_HOOK", h)
        m.get_axon_ntff_profile_hook = lambda: m._HOOK
        sys.modules["antenv.axon_hooks"] = m
        import antenv

        antenv.axon_hooks = m
    from antenv.axon_hooks import (
        get_axon_ntff_profile_hook,
        set_axon_ntff_profile_hook,
    )

    if get_axon_ntff_profile_hook() is None:
        import trn_agent_boot.trn_boot as tb

        set_axon_ntff_profile_hook(
            tb._ntff_profile_via_ctypes("/opt/axon/libaxon_pjrt.so")
        )


def kernel(x, Wq, bq, Wk, bk, Wv, bv, Wo, bo, _trace=False):
    x, Wq, bq, Wk, bk, Wv, bv, Wo, bo = (
        np.asarray(a, dtype=np.float32) for a in (x, Wq, bq, Wk, bk, Wv, bv, Wo, bo)
    )
    if "nc" not in _CACHE:
        _CACHE["nc"] = _build()
    nc = _CACHE["nc"]
    in_maps = _prep_inputs(x, Wq, bq, Wk, bk, Wv, bv, Wo, bo)
    kwargs = {}
    if _trace:
        _install_ntff_hook()
        kwargs = dict(trace=True, trace_cores=[0])
    res = run_bass_kernel_spmd(nc, in_maps, core_ids=list(range(8)), **kwargs)
    _CACHE["last_result"] = res
    y = np.zeros((TOK, D), dtype=np.float64)
    for r in res.results:
        y += r["y"].astype(np.float64) / r["l"].astype(np.float64)[:, None]
    y += bo[None, :]
    return y.astype(np.float32).reshape(B, S, D)


# revision 12
# speedup vs baseline: 1.0368x; 1.0368x over previous
"""Causal self-attention (B=2, S=4096, D=512, H=8) on 8 Trainium2 NeuronCores.

Sharding: tensor-parallel over heads. Core h computes head h for both batch
elements: QKV projections for its head, causal flash attention, and its
partial (unnormalized) o_proj contribution y_h = U_h @ Wo[h*64:(h+1)*64, :]
plus the per-query softmax denominators L_h. The host computes
sum_h(y_h / L_h) + bo.

v2 layout (hd = 64, S = 4096, 32 k-tiles of 128 per batch):
  - All 16 xt tiles [128, 4, 512] (bf16, host-pretransposed x) are resident
    in SBUF and prefetched on the Sync queue at t0.
  - Projections per 512-token block: Wq -> q_ps[0:64], [Wk|Wv] -> kv_ps
    (K.T rows 0:64, V.T rows 64:128). Bias-adds write bf16 Q.T/K.T straight
    to partition-base-0 tiles (no duplication / zero-padding DMAs); V.T is
    PE-transposed (identity at base 64) into V-natural blocks vp=[V|ones].
  - Scores: K=64 matmuls lhsT=K.T[64,128], rhs=Q.T[64,512] -> st psum
    [128, 2, 512]; one ACT exp (scale) -> P.T bf16; diagonal chunks get a
    0/1 causal mask multiply on DVE.
  - AV (bf16): U'[65, 512] += V'_kt.T @ P.T_kt; row 64 accumulates L.
  - o_proj: K=65 matmuls lhsT=U'[65,128] (fp32r), rhs=Wo_h' [65,512]
    (row 64 zeroed) -> y [128, 512] fp32, cast bf16, DMA'd out with L.
  - Engine split: PE matmuls; ACT exp + U' psum->sbuf copies; DVE bias-adds
    + causal masks; Pool (gpsimd) vp/y copies + all output DMAs (software
    DGE); Sync all input DMAs. Emission is software-pipelined: the next
    block's projection matmuls are issued inside the previous block's last
    exp window to keep the PE p-state hot.
"""

import sys

for _p in ("/opt/trn_rl_repo", "/root/.axon_site/_ro/trn_rl_repo"):
    if _p not in sys.path:
        sys.path.insert(0, _p)

import numpy as np

import concourse.bass as bass
import concourse.mybir as mybir
import concourse.tile as tile
from concourse import bacc
from concourse.bass_utils import run_bass_kernel_spmd

B = 2
S = 4096
D = 512
H = 8
HD = 64
TOK = B * S          # 8192
NKT = S // 128       # 32 k-tiles per batch
NBLK = 16            # 512-token blocks over both batches
SCALE = HD ** -0.5

F32 = mybir.dt.float32
F32R = mybir.dt.float32r
BF16 = mybir.dt.bfloat16

_CACHE = {}


def _build():
    nc = bacc.Bacc("TRN2", target_bir_lowering=False, debug=False, num_devices=8)

    # xt pre-tiled on host: [NBLK*128, 4, 512] so each block's DMA is one
    # fully contiguous 512 KiB read (4 KiB per partition line).
    xt_d = nc.dram_tensor("xt", [NBLK * 128, 4, 512], BF16, kind="ExternalInput")
    # wpack: wq [4*64] | wkv [4*128] | bq [1] | bkv [1] | mask [4*512]
    #        | ident [64] | ones [NKT], all bf16, one DMA.
    WPACK = 256 + 512 + 1 + 1 + 2048 + 64 + NKT
    wpack_d = nc.dram_tensor("wpack", [128, WPACK], BF16, kind="ExternalInput")
    wo_d = nc.dram_tensor("wo", [65, D], F32R, kind="ExternalInput")
    y_d = nc.dram_tensor("y", [TOK, D], BF16, kind="ExternalOutput")
    l_d = nc.dram_tensor("l", [TOK], F32R, kind="ExternalOutput")

    blocks = [(b, tb) for b in range(B) for tb in range(8)]

    with tile.TileContext(nc) as tc:
        import contextlib

        with contextlib.ExitStack() as ctx:
            singles = ctx.enter_context(tc.tile_pool(name="singles", bufs=1))
            ptpool = ctx.enter_context(tc.tile_pool(name="pt", bufs=4))
            upool = ctx.enter_context(tc.tile_pool(name="usb", bufs=2))
            ypool = ctx.enter_context(tc.tile_pool(name="ysb", bufs=4))
            vtpool = ctx.enter_context(tc.tile_pool(name="vt", bufs=2))

            ps_st = ctx.enter_context(
                tc.tile_pool(name="ps_st", bufs=2, space="PSUM")
            )
            ps_u = ctx.enter_context(tc.tile_pool(name="ps_u", bufs=1, space="PSUM"))
            ps_misc = ctx.enter_context(
                tc.tile_pool(name="ps_misc", bufs=3, space="PSUM")
            )

            # --- constants / weights (issue order = load order on Sync) ---
            wpack_sb = singles.tile([128, WPACK], BF16)
            nc.sync.dma_start(out=wpack_sb, in_=wpack_d.ap())
            wq_sb = wpack_sb[:, 0:256].rearrange("p (c m) -> p c m", c=4)
            wkv_sb = wpack_sb[:, 256:768].rearrange("p (c m) -> p c m", c=4)
            bias_sb = singles.tile([128, 2], F32)
            nc.vector.tensor_copy(bias_sb, wpack_sb[:, 768:770])
            bq_sb = bias_sb[:, 0:1]
            bkv_sb = bias_sb[:, 1:2]
            mask_sb = wpack_sb[:, 770:2818].rearrange("p (d m) -> p d m", d=4)
            identb = wpack_sb[:, 2818:2882]
            onescol = wpack_sb[:, 2882 : 2882 + NKT]

            xts = [
                singles.tile([128, 4, 512], BF16, tag=f"xt{i}", name=f"xt{i}")
                for i in range(NBLK)
            ]
            nc.sync.dma_start(out=xts[0], in_=xt_d.ap()[0:128, :, :])

            wo_sb = singles.tile([65, D], F32R)
            nc.sync.dma_start(out=wo_sb, in_=wo_d.ap())
            for i in range(1, NBLK):
                nc.sync.dma_start(
                    out=xts[i], in_=xt_d.ap()[i * 128 : (i + 1) * 128, :, :]
                )

            # --- persistent per-batch activation buffers ---------------
            qt = [
                singles.tile([HD, S], BF16, tag=f"qt_{b}", name=f"qt_{b}")
                for b in range(B)
            ]
            kt = [
                singles.tile([HD, S], BF16, tag=f"kt_{b}", name=f"kt_{b}")
                for b in range(B)
            ]
            vp = [
                singles.tile([128, NKT * 65], BF16, tag=f"vp_{b}", name=f"vp_{b}")
                for b in range(B)
            ]
            for b in range(B):
                nc.vector.tensor_copy(
                    vp[b].rearrange("p (t c) -> p t c", c=65)[:, :, 64:65],
                    onescol.rearrange("p (t c) -> p t c", c=1),
                )

            def proj_mm(i):
                """Projection matmuls (PE only) for block i; kv first so the
                V transposes and K add unblock before the q group retires."""
                b, tb = blocks[i]
                xt_sb = xts[i]
                kv_ps = ps_misc.tile([128, 512], F32, tag="m")
                for c in range(4):
                    nc.tensor.matmul(
                        kv_ps,
                        wkv_sb[:, c, :],
                        xt_sb[:, c, :],
                        start=(c == 0),
                        stop=(c == 3),
                    )
                q_ps = ps_misc.tile([128, 512], F32, tag="m")
                for c in range(4):
                    nc.tensor.matmul(
                        q_ps[0:HD, :],
                        wq_sb[:, c, :],
                        xt_sb[:, c, :],
                        start=(c == 0),
                        stop=(c == 3),
                    )
                return q_ps, kv_ps

            def proj_adds(i, q_ps, kv_ps):
                """Bias-adds (DVE); returns vt_sb for the transpose step."""
                b, tb = blocks[i]
                cols = slice(tb * 512, (tb + 1) * 512)
                vt_sb = vtpool.tile([128, 512], BF16, tag="vt")
                nc.vector.tensor_scalar_add(
                    vt_sb[64:128, :], kv_ps[64:128, :], bkv_sb[64:128, 0:1]
                )
                nc.vector.tensor_scalar_add(
                    qt[b][:, cols], q_ps[0:HD, :], bq_sb[0:HD, 0:1]
                )
                nc.vector.tensor_scalar_add(
                    kt[b][:, cols], kv_ps[0:HD, :], bkv_sb[0:HD, 0:1]
                )
                return vt_sb

            def proj_vp(i, vt_sb):
                """V transposes (PE) + vp copies (DVE)."""
                b, tb = blocks[i]
                for j in range(4):
                    ktile = tb * 4 + j
                    vtr_ps = ps_misc.tile([128, HD], BF16, tag="m")
                    nc.tensor.transpose(
                        vtr_ps,
                        vt_sb[64:128, j * 128 : (j + 1) * 128],
                        identb[64:128, :],
                    )
                    nc.vector.tensor_copy(
                        vp[b][:, ktile * 65 : ktile * 65 + 64], vtr_ps
                    )

            def make_emit_av(b, qb, u_ps, n_chunks):
                def emit_av(pt, j):
                    for j2 in range(2):
                        ktile = 2 * j + j2
                        dj = ktile - 4 * qb
                        c0 = 128 * dj if dj > 0 else 0
                        nc.tensor.matmul(
                            u_ps[:, c0:512],
                            vp[b][:, ktile * 65 : ktile * 65 + 65],
                            pt[:, j2, c0:512],
                            start=(ktile == 0),
                            stop=(ktile == 2 * n_chunks - 1),
                            skip_group_check=True,
                        )

                return emit_av

            def attn_chunks(i):
                """ST/exp/mask chunks + all AVs except the last pair.

                Diagonal k-tile dj (0..3) only contributes to queries with
                column >= 128*dj in the q-block; scores/exp/AV are trimmed
                to that column range."""
                b, qb = blocks[i]
                q0 = qb * 512
                u_ps = ps_u.tile([65, 512], F32, tag="u")
                n_chunks = 2 * (qb + 1)  # chunks of 2 k-tiles
                emit_av = make_emit_av(b, qb, u_ps, n_chunks)

                prev_pt = None
                for j in range(n_chunks):
                    st = ps_st.tile([128, 2, 512], F32, tag="st")
                    for j2 in range(2):
                        ktile = 2 * j + j2
                        dj = ktile - 4 * qb
                        c0 = 128 * dj if dj > 0 else 0
                        nc.tensor.matmul(
                            st[:, j2, c0:512],
                            kt[b][:, ktile * 128 : (ktile + 1) * 128],
                            qt[b][:, q0 + c0 : q0 + 512],
                            start=True,
                            stop=True,
                        )
                    pt = ptpool.tile([128, 2, 512], BF16, tag="pt")
                    c0e = 256 if j == n_chunks - 1 else 0
                    nc.scalar.activation(
                        pt[:, :, c0e:512],
                        st[:, :, c0e:512],
                        mybir.ActivationFunctionType.Exp,
                        scale=SCALE,
                    )
                    if j >= n_chunks - 2:  # diagonal chunks: causal mask
                        d0 = (j % 2) * 2
                        nc.vector.tensor_mul(
                            pt[:, :, c0e:512],
                            pt[:, :, c0e:512],
                            mask_sb[:, d0 : d0 + 2, c0e:512],
                        )
                    if prev_pt is not None:
                        emit_av(prev_pt, j - 1)
                    prev_pt = pt
                return u_ps, prev_pt, emit_av, n_chunks

            def attn_tail_mm(i, u_ps, last_pt, emit_av, n_chunks):
                """Last AV pair, U' drain (ACT), o_proj matmuls, L out."""
                b, qb = blocks[i]
                emit_av(last_pt, n_chunks - 1)

                u_sb = upool.tile([65, 512], F32R, tag="u")
                nc.scalar.copy(u_sb, u_ps)

                row0 = b * S + qb * 512
                nc.gpsimd.dma_start(
                    out=l_d.ap()[row0 : row0 + 512].rearrange("(p c) -> p c", p=1),
                    in_=u_sb[64:65, :],
                )

                # y = U'.T @ Wo_h' (unnormalized); K=65, wo row 64 = 0
                y_pss = []
                for j2 in range(4):
                    y_ps = ps_misc.tile([128, 512], F32, tag="m")
                    nc.tensor.matmul(
                        y_ps,
                        u_sb[:, j2 * 128 : (j2 + 1) * 128],
                        wo_sb,
                        start=True,
                        stop=True,
                    )
                    y_pss.append(y_ps)
                return row0, y_pss

            def attn_tail_out(row0, y_pss):
                """y psum->sbuf casts (DVE) + output DMAs (GpSimd swdge)."""
                for j2, y_ps in enumerate(y_pss):
                    y_sb = ypool.tile([128, 512], BF16, tag="y")
                    nc.vector.tensor_copy(y_sb, y_ps)
                    r0 = row0 + j2 * 128
                    nc.gpsimd.dma_start(out=y_d.ap()[r0 : r0 + 128, :], in_=y_sb)

            # Zero the pt pool once: trimmed exp calls leave stale columns
            # that the causal mask multiplies by 0 — NaN*0 must not happen.
            for _ in range(4):
                ptz = ptpool.tile([128, 2, 512], BF16, tag="pt")
                nc.gpsimd.memset(ptz, 0)

            # Software pipeline: proj MMs of block i+1 land inside block i's
            # last exp window; block i's tail (o_proj etc.) follows them.
            q_ps, kv_ps = proj_mm(0)
            proj_vp(0, proj_adds(0, q_ps, kv_ps))
            for i in range(NBLK):
                chunk_state = attn_chunks(i)
                if i + 1 < NBLK:
                    q_ps, kv_ps = proj_mm(i + 1)
                row0, y_pss = attn_tail_mm(i, *chunk_state)
                if i + 1 < NBLK:
                    vt_sb = proj_adds(i + 1, q_ps, kv_ps)
                    if blocks[i + 1][1] == 0:
                        # batch start: attn(i+1) needs its own vp tiles at
                        # chunk 0 — emit them ahead of the y drains.
                        proj_vp(i + 1, vt_sb)
                        attn_tail_out(row0, y_pss)
                    else:
                        attn_tail_out(row0, y_pss)
                        proj_vp(i + 1, vt_sb)
                else:
                    attn_tail_out(row0, y_pss)

    nc.compile()
    return nc


def _prep_inputs(x, Wq, bq, Wk, bk, Wv, bv, Wo, bo):
    import ml_dtypes

    bf16 = ml_dtypes.bfloat16
    # xt pre-tiled: [NBLK, 128, 4, 512] -> [NBLK*128, 4, 512] contiguous
    xt = x.reshape(TOK, D).T  # [512, 8192]
    xtt = np.empty((NBLK, 128, 4, 512), dtype=np.float32)
    for i in range(NBLK):
        xtt[i] = xt[:, i * 512 : (i + 1) * 512].reshape(4, 128, 512).transpose(
            1, 0, 2
        )
    xtt = np.ascontiguousarray(xtt.reshape(NBLK * 128, 4, 512)).astype(bf16)

    mask = np.zeros((128, 4, 512), dtype=np.float32)
    p = np.arange(128)[:, None]
    c = np.arange(512)[None, :]
    for d in range(4):
        mask[:, d, :] = (p + 128 * d <= c).astype(np.float32)
    identb = np.zeros((128, 64), dtype=np.float32)
    identb[64:128, :] = np.eye(64, dtype=np.float32)

    def pack_w(w):  # [512, M] -> [128, 4*M] in "(c p) m" tile order
        m = w.shape[1]
        return w.reshape(4, 128, m).transpose(1, 0, 2).reshape(128, 4 * m)

    in_maps = []
    for h in range(H):
        hs = slice(h * HD, (h + 1) * HD)
        wo_h = np.concatenate(
            [Wo[hs, :], np.zeros((1, D), dtype=np.float32)], axis=0
        ).astype(np.float32)
        wpack = np.concatenate(
            [
                pack_w(Wq[:, hs]),                                   # 256
                pack_w(np.concatenate([Wk[:, hs], Wv[:, hs]], 1)),   # 512
                np.concatenate([bq[hs], bq[hs]]).reshape(128, 1),    # 1
                np.concatenate([bk[hs], bv[hs]]).reshape(128, 1),    # 1
                mask.reshape(128, 2048),                             # 2048
                identb,                                              # 64
                np.ones((128, NKT), dtype=np.float32),               # NKT
            ],
            axis=1,
        ).astype(bf16)
        in_maps.append({"xt": xtt, "wpack": wpack, "wo": wo_h})
    return in_maps


def _install_ntff_hook():
    """Register the axon NTFF profiling hook (test-only plumbing)."""
    import types

    try:
        from antenv.axon_hooks import set_axon_ntff_profile_hook  # noqa: F401
    except ImportError:
        m = types.ModuleType("antenv.axon_hooks")
        m._HOOK = None
        m.set_axon_ntff_profile_hook = lambda h: setattr(m, "_HOOK", h)
        m.get_axon_ntff_profile_hook = lambda: m._HOOK
        sys.modules["antenv.axon_hooks"] = m
        import antenv

        antenv.axon_hooks = m
    from antenv.axon_hooks import (
        get_axon_ntff_profile_hook,
        set_axon_ntff_profile_hook,
    )

    if get_axon_ntff_profile_hook() is None:
        import trn_agent_boot.trn_boot as tb

        set_axon_ntff_profile_hook(
            tb._ntff_profile_via_ctypes("/opt/axon/libaxon_pjrt.so")
        )


def kernel(x, Wq, bq, Wk, bk, Wv, bv, Wo, bo, _trace=False):
    x, Wq, bq, Wk, bk, Wv, bv, Wo, bo = (
        np.asarray(a, dtype=np.float32) for a in (x, Wq, bq, Wk, bk, Wv, bv, Wo, bo)
    )
    if "nc" not in _CACHE:
        _CACHE["nc"] = _build()
    nc = _CACHE["nc"]
    in_maps = _prep_inputs(x, Wq, bq, Wk, bk, Wv, bv, Wo, bo)
    kwargs = {}
    if _trace:
        _install_ntff_hook()
        kwargs = dict(trace=True, trace_cores=[0])
    res = run_bass_kernel_spmd(nc, in_maps, core_ids=list(range(8)), **kwargs)
    _CACHE["last_result"] = res
    y = np.zeros((TOK, D), dtype=np.float64)
    for r in res.results:
        y += r["y"].astype(np.float64) / r["l"].astype(np.float64)[:, None]
    y += bo[None, :]
    return y.astype(np.float32).reshape(B, S, D)
